# revision 5
# baseline (speedup 1.0000x reference)
"""Trainium2 Bass kernel for nn_ColRepeatCausalLinear.

Math: reference computes out = x @ W + bias with
    W[s, t] = v[t] * d^(t-s)  for t >= s, else 0,   d = clip(decay_value, 0.9, 1)
which factorizes as a decayed prefix scan along S:
    y[b, e, t] = d * y[b, e, t-1] + x[b, e, t]
    out[b, e, t] = v[t] * y[b, e, t] + bias[t]
i.e. O(B*E*S) work instead of the O(B*E*S^2) dense matmul.

Mapping: data-parallel over B across 8 NeuronCores (x[b] per core, params
replicated). Per core the kernel is DMA-bound (the scan+scale is one fused
Vector-engine op per 128x2048 tile), so I/O is done in fp16: the host casts
x/v to fp16 (quantization ~5e-4 L2 rel err, budget is 2e-2), the device
scans in fp32 internally (DVE ports upconvert), and the fp16 result is
upcast on the host. That halves HBM traffic: 8.4 MB/core instead of 16.8.
All 16+1 tiles live in SBUF simultaneously (68 KiB/partition of 208), so
every load issues at t=0 with no buffer-reuse (WAR) stalls, spread over the
three DGE rings (SP + ACT HWDGE, GpSimd SWDGE).

Hardcoded problem shapes: x (8, 1024, 2048) f32, weight (1, 2048),
bias (2048,), decay_value (1,).
"""

import numpy as np

import concourse.bacc as bacc
import concourse.mybir as mybir
from concourse.tile import TileContext
from concourse.bass_utils import run_bass_kernel_spmd

B, E, S = 8, 1024, 2048
P = 128
N_CORES = 8
F32 = mybir.dt.float32
F16 = mybir.dt.float16

_cache = {}

# Fused custom DVE op: out[p,k] = (sum_{j<=k} x[p,j]) * v[p,k] — the whole
# d=1 kernel body in ONE Vector-engine instruction (the stock path needs a
# 2-cyc/elem TensorTensorScan plus a 1-cyc/elem tensor_mul). Registered at
# runtime into dve_ops.OPS; sha self-pinned since this op isn't in-tree.
_FUSED_OP = None
try:
    from concourse import dve_ops as _dops
    from concourse.dve_spec import AluOp as _AluOp, Spec as _Spec
    from concourse.dve_spec import Src0 as _Src0, Src1 as _Src1, scan as _scan
    from concourse.dve_spec import lower as _lower
    from concourse.dve_uop import DveOpSpec as _DveOpSpec

    _FUSED_NAME = "CUMSUM_VSCALE_ANT"
    if _FUSED_NAME in _dops._SUB_OPCODE_FOR_NAME:
        _FUSED_OP = next(o for o in _dops.OPS if o.name == _FUSED_NAME)
    else:
        _fspec = _Spec(body=_scan(_AluOp.ADD, _Src0) * _Src1)
        _row = _dops._CUSTOM_DVE_ROW_BASE + len(_dops.OPS)
        assert _row < 0x20
        _dops._SUB_OPCODE_FOR_NAME[_FUSED_NAME] = _row
        _sha = {}
        for _ver in ("v3", "v4"):
            try:
                _sha[_ver] = _DveOpSpec(
                    name=_FUSED_NAME,
                    opcode=_row,
                    uops=_lower(_fspec, ver=_ver),
                    rd1_en=_dops.has_src1(_fspec),
                ).sha(_ver)
            except Exception:
                pass
        _FUSED_OP = _dops.DveOp(_FUSED_NAME, _fspec, subdim=False, uops_sha=_sha)
        _dops.OPS.append(_FUSED_OP)
        _dops.CUSTOM_DVE_SPECS[_FUSED_NAME] = _fspec
except Exception:
    _FUSED_OP = None


def _build_fp16():
    """Fast path: d == 1, no bias, fp16 I/O, fused scan*v DVE op."""
    nc = bacc.Bacc(
        "TRN2",
        target_bir_lowering=False,
        debug=False,
        enable_asserts=False,
    )
    x = nc.dram_tensor("x", [E, S], F16, kind="ExternalInput").ap()
    vb_dram = nc.dram_tensor("vb", [P, S], F16, kind="ExternalInput").ap()
    out = nc.dram_tensor("out", [E, S], F16, kind="ExternalOutput").ap()

    n_tiles = E // P
    with TileContext(nc) as tc:
        with (
            tc.tile_pool(name="const", bufs=1) as cpool,
            tc.tile_pool(name="xs", bufs=1) as xpool,
            tc.tile_pool(name="os", bufs=1) as opool,
        ):
            vb = cpool.tile([P, S], F16)
            nc.scalar.dma_start(out=vb[:], in_=vb_dram)
            xts = []
            # Tile 0 split across both HWDGE rings so scan0 starts ASAP;
            # remaining loads alternate rings in tile order so loads
            # complete in the order the DVE consumes them.
            xt0 = xpool.tile([P, S], F16, name="xt0", bufs=1)
            nc.sync.dma_start(out=xt0[: P // 2, :], in_=x[: P // 2, :])
            nc.scalar.dma_start(
                out=xt0[P // 2 :, :], in_=x[P // 2 : P, :]
            )
            xts.append(xt0)
            for i in range(1, n_tiles):
                xt = xpool.tile([P, S], F16, name=f"xt{i}", bufs=1)
                (nc.sync if i % 2 else nc.scalar).dma_start(
                    out=xt[:], in_=x[i * P : (i + 1) * P, :]
                )
                xts.append(xt)
            # Stores alternate rings, queued after the loads (FIFO per
            # ring), so a store blocked on its scan never delays a load.
            for i in range(n_tiles):
                ot = opool.tile([P, S], F16, name=f"ot{i}", bufs=1)
                nc.vector._custom_dve(
                    _FUSED_OP, out=ot[:], in0=xts[i][:], in1=vb[:]
                )
                (nc.sync if i % 2 else nc.scalar).dma_start(
                    out=out[i * P : (i + 1) * P, :], in_=ot[:]
                )
    nc.compile()
    return nc


def _build(d: float, has_bias: bool):
    """General path (any d in [0.9, 1], optional bias), fp32 throughout."""
    nc = bacc.Bacc(
        "TRN2",
        target_bir_lowering=False,
        debug=False,
        enable_asserts=False,
    )
    x = nc.dram_tensor("x", [E, S], F32, kind="ExternalInput").ap()
    vb_dram = nc.dram_tensor("vb", [P, S], F32, kind="ExternalInput").ap()
    bias_dram = None
    if has_bias:
        bias_dram = nc.dram_tensor("biasb", [P, S], F32, kind="ExternalInput").ap()
    out = nc.dram_tensor("out", [E, S], F32, kind="ExternalOutput").ap()

    with TileContext(nc) as tc:
        with (
            tc.tile_pool(name="const", bufs=1) as cpool,
            tc.tile_pool(name="xs", bufs=6) as xpool,
            tc.tile_pool(name="ys", bufs=2) as ypool,
            tc.tile_pool(name="os", bufs=4) as opool,
        ):
            # decay operand: [P, 1] column broadcast along the free axis
            dtile = cpool.tile([P, 1], F32)
            nc.gpsimd.memset(dtile[:], d)
            dbcast = dtile[:].broadcast_to([P, S])
            H = S // 2
            n_tiles = E // P
            vb = cpool.tile([P, S], F32)
            if has_bias:
                bb = cpool.tile([P, S], F32)
            rings = [nc.sync, nc.scalar, nc.gpsimd]
            rr = [0]

            def ring():
                r = rings[rr[0] % 3]
                rr[0] += 1
                return r

            for i in range(n_tiles):
                xt = xpool.tile([P, S], F32)
                ring().dma_start(out=xt[:], in_=x[i * P : (i + 1) * P, :])
                if i == 0:
                    nc.scalar.dma_start(out=vb[:], in_=vb_dram)
                    if has_bias:
                        nc.scalar.dma_start(out=bb[:], in_=bias_dram)
                yt = ypool.tile([P, S], F32)
                nc.vector.tensor_tensor_scan(
                    yt[:], dbcast, xt[:],
                    0.0, mybir.AluOpType.mult, mybir.AluOpType.add,
                )
                ot = opool.tile([P, S], F32)
                if i == n_tiles - 1:
                    # Last tile: split the mult so each half-store (on its
                    # own HWDGE ring) starts as soon as its half is ready.
                    nc.vector.tensor_mul(ot[:, :H], yt[:, :H], vb[:, :H])
                    if has_bias:
                        nc.vector.tensor_add(ot[:, :H], ot[:, :H], bb[:, :H])
                    nc.scalar.dma_start(
                        out=out[i * P : (i + 1) * P, :H], in_=ot[:, :H]
                    )
                    nc.vector.tensor_mul(ot[:, H:], yt[:, H:], vb[:, H:])
                    if has_bias:
                        nc.vector.tensor_add(ot[:, H:], ot[:, H:], bb[:, H:])
                    nc.sync.dma_start(
                        out=out[i * P : (i + 1) * P, H:], in_=ot[:, H:]
                    )
                else:
                    nc.vector.tensor_mul(ot[:], yt[:], vb[:])
                    if has_bias:
                        nc.vector.tensor_add(ot[:], ot[:], bb[:])
                    nc.scalar.dma_start(out=out[i * P : (i + 1) * P, :], in_=ot[:])
    nc.compile()
    return nc


def _run(x, weight, bias, decay_value, trace=False):
    x = np.asarray(x, dtype=np.float32)
    weight = np.asarray(weight, dtype=np.float32)
    bias = np.asarray(bias, dtype=np.float32)
    decay_value = np.asarray(decay_value)
    assert x.shape == (B, E, S), x.shape

    # DECAY_CONSTANT = 1.0 in the reference; exponent is (t - s) / 1.0.
    d = float(np.clip(np.float64(decay_value.reshape(-1)[0]), 0.9, 1.0))
    has_bias = bool(np.any(bias))

    if d == 1.0 and not has_bias and _FUSED_OP is not None:
        if "fp16" not in _cache:
            _cache["fp16"] = _build_fp16()
        nc = _cache["fp16"]
        vb = np.ascontiguousarray(
            np.broadcast_to(weight.reshape(1, S).astype(np.float16), (P, S))
        )
        in_maps = [
            {"x": x[b].astype(np.float16), "vb": vb} for b in range(N_CORES)
        ]
        res = run_bass_kernel_spmd(
            nc, in_maps, core_ids=list(range(N_CORES)), trace=trace
        )
        out = np.stack(
            [r["out"].astype(np.float32) for r in res.results], axis=0
        )
        return out, res

    key = (d, has_bias)
    if key not in _cache:
        _cache[key] = _build(d, has_bias)
    nc = _cache[key]

    vb = np.ascontiguousarray(
        np.broadcast_to(weight.reshape(1, S), (P, S)), dtype=np.float32
    )
    bb = None
    if has_bias:
        bb = np.ascontiguousarray(
            np.broadcast_to(bias.reshape(1, S), (P, S)), dtype=np.float32
        )

    in_maps = []
    for b in range(N_CORES):
        m = {"x": np.ascontiguousarray(x[b]), "vb": vb}
        if has_bias:
            m["biasb"] = bb
        in_maps.append(m)

    res = run_bass_kernel_spmd(
        nc, in_maps, core_ids=list(range(N_CORES)), trace=trace
    )
    out = np.stack([r["out"] for r in res.results], axis=0)
    return out, res


def kernel(x, weight, bias, decay_value):
    out, _ = _run(x, weight, bias, decay_value)
    return out
